# revision 4
# baseline (speedup 1.0000x reference)
"""DLinear layer (nn_DLinearLayer) TRN2 Bass kernel — single-GEMM formulation.

Math (reference):
    trend[b,t,f]  = avgpool2(x)[b,t,f] = 0.5*(x[t]+x[t+1]), last: x[T-1]
    resid         = x - trend
    out[b,n,f]    = trend[:,:,f] @ trend_W[f] + trend_b[f,n]
                  + resid[:,:,f] @ residual_W[f] + residual_b[f,n]

Identity used here: with B[t] = x[t+1] (B[T-1] = x[T-1]),
    trend = (x+B)/2, resid = (x-B)/2, and shift(x) @ V == x @ V' where
    V'[s] = V[s-1] (V'[0] = 0, V'[T-1] += V[T-1]), so the layer folds to
    ONE GEMM per feature:
    out[:, :, f] = x[:, :, f] @ Weff[f] + (tb+rb)[f]
    Weff[f] = (Wt[f]+Wr[f])/2 + shift_down((Wt[f]-Wr[f])/2)  [+ last-row fixup]
Weff precomputed on host; bias added on host ([F,N] broadcast — zero HW
cost). Halves device FLOPs and weight DMA vs the 2-GEMM form.

Sharding: feature-expert — core k owns features {2k, 2k+1}; every weight
byte is read exactly once across the system.

Perf notes (from perfetto of v1):
  * DMA channels cost ~13ns + ~35ns/KB per partition-line descriptor and
    each dma_start costs ~650ns of issuing-sequencer time -> use FEW
    dma_starts with LONG contiguous lines. W is host-packed partition-
    major ([FL,128,TC,N]) so a 2-chunk "group" load moves 4KB/partition.
  * PE streams ~1 col/cycle (267ns per 128x128x512 bf16 matmul) -> 64
    matmuls ~= 17us is the compute floor; DMA (~12us) hides under it.
  * Matmul order (c, b, h) reuses the stationary x tile across the two
    N-halves, halving LDWEIGHTS.
  * Drain tail: last feature's casts fan out over DVE/ACT/GpSimd.
Dtypes: x / Weff / out bf16 (PSUM accumulates fp32): rel-l2 ~2.9e-3.
"""

import numpy as np

import concourse.bass as bass
import concourse.mybir as mybir
import concourse.tile as tile
from concourse.bass_utils import run_bass_kernel_spmd

F, B, T, N = 16, 256, 1024, 1024
NCORES = 8
FL = F // NCORES          # features per core
TC = T // 128             # contraction chunks (t on SBUF partitions)
NB = B // 128             # output partition tiles
NH = N // 512             # output free-dim halves (one PSUM bank each)
NG = TC // 2              # W DMA groups (2 chunks = 4KB lines) per feature
HALF = TC // 2
F32 = mybir.dt.float32
BF16 = mybir.dt.bfloat16


def _split_multi_waits(nc):
    """This container's walrus build accepts at most ONE sem wait per
    instruction ("Too many sync wait commands" in CoreV3Gen setupSyncWait).
    Tile emits 2+. Move excess waits onto nofuse NoOps placed immediately
    before the owning instruction on the same engine: engines execute their
    stream in order, so semantics are unchanged."""
    for fn in nc.m.functions:
        for blk in fn.blocks:
            out = []
            for inst in blk.instructions:
                si = inst.sync_info
                if si is not None and si.on_wait and len(si.on_wait) > 1:
                    waits = list(si.on_wait)
                    for j, w in enumerate(waits[:-1]):
                        out.append(mybir.InstNoOp(
                            name=f"{inst.name}-ws{j}",
                            engine=inst.engine,
                            bass_nofuse=True,
                            sync_info=mybir.SyncInfo(on_wait=[w], on_update=[]),
                        ))
                    si.on_wait = [waits[-1]]
                out.append(inst)
            blk.instructions[:] = out


def _build():
    nc = bass.Bass(trn_type="TRN2")

    # partition-major layouts: per-partition DRAM lines are long/contiguous
    #   xP[f, p, c, b] = x[b, c*128+p, f]          (line = TC*B*2  = 4KB)
    #   wP[f, p, c, n] = Weff[f, c*128+p, n]       (line = TC*N*2 = 16KB)
    x_d = nc.dram_tensor("xP", [FL, 128, TC, B], BF16, kind="ExternalInput")
    w_d = nc.dram_tensor("wP", [FL, 128, TC, N], BF16, kind="ExternalInput")
    out_d = nc.dram_tensor("out", [FL, B, N], BF16, kind="ExternalOutput")

    with tile.TileContext(nc) as tc:
        with (
            tc.tile_pool(name="xp", bufs=2) as xp,
            tc.tile_pool(name="wp", bufs=FL * NG) as wp,
            tc.tile_pool(name="ob", bufs=FL * NB) as obp,
            tc.tile_pool(name="ps", bufs=8, space="PSUM") as psp,
        ):
            q = [nc.sync, nc.scalar]   # the two HWDGE queues on TRN2

            xt = {f: xp.tile([128, TC, B], BF16, tag="x", name=f"x{f}")
                  for f in range(FL)}
            wt = {(f, g): wp.tile([128, 2, N], BF16, tag="w", name=f"w{f}_{g}")
                  for f in range(FL) for g in range(NG)}

            # ---- loads: issue order == consumption order, alternating
            # queues; x tiles split by chunk-half across both queues. The
            # very last W group is split per-chunk so the final matmuls
            # start half a transfer earlier.
            def x_halves(f):
                q[0].dma_start(xt[f][:, 0:HALF, :], x_d[f, :, 0:HALF, :])
                q[1].dma_start(xt[f][:, HALF:TC, :], x_d[f, :, HALF:TC, :])

            x_halves(0)
            for g in range(NG):
                q[g % 2].dma_start(wt[0, g][:], w_d[0, :, 2 * g:2 * g + 2, :])
                if g == 1:
                    x_halves(1)
            for g in range(NG - 1):
                q[g % 2].dma_start(wt[1, g][:], w_d[1, :, 2 * g:2 * g + 2, :])
            for j in range(2):
                q[(NG - 1 + j) % 2].dma_start(
                    wt[1, NG - 1][:, j, :],
                    w_d[1, :, 2 * (NG - 1) + j, :])

            # ---- GEMM chains: psum[b,h] per feature accumulates over the
            # 8 t-chunks; (c, b, h) order reuses the stationary x[c,b] tile
            # for both N-halves (halves LDWEIGHTS).
            drains = []   # (f, b, ot) in completion order
            for f in range(FL):
                ps = {(b, h): psp.tile([128, 512], F32, tag="ps",
                                       name=f"ps{f}_{b}_{h}")
                      for b in range(NB) for h in range(NH)}
                for c in range(TC):
                    g, j = divmod(c, 2)
                    for b in range(NB):
                        for h in range(NH):
                            ns = slice(h * 512, (h + 1) * 512)
                            nc.tensor.matmul(
                                ps[b, h][:],
                                xt[f][:, c, b * 128:(b + 1) * 128],
                                wt[f, g][:, j, ns],
                                start=(c == 0), stop=(c == TC - 1))
                # drain: cast fp32 psum -> bf16 sbuf, one [128, N] tile per
                # (f, b); mid-run casts on DVE (idle), last-feature casts
                # fan out DVE/ACT/GpSimd to shorten the tail.
                for b in range(NB):
                    ot = obp.tile([128, N], BF16, tag="o", name=f"o{f}_{b}")
                    for h in range(NH):
                        ns = slice(h * 512, (h + 1) * 512)
                        if f < FL - 1 or (b * NH + h) % 2 == 0:
                            nc.vector.tensor_copy(ot[:, ns], ps[b, h][:])
                        else:
                            nc.scalar.copy(ot[:, ns], ps[b, h][:])
                    drains.append((f, b, ot))

            # ---- stores: one dma_start per (f, b) [128 x 2KB lines].
            # f0 rides SWDGE mid-run; f1 uses the by-then-idle HWDGE queues.
            for f, b, ot in drains:
                bs = slice(b * 128, (b + 1) * 128)
                if f < FL - 1:
                    nc.gpsimd.dma_start(out_d[f, bs, :], ot[:])
                else:
                    q[b % 2].dma_start(out_d[f, bs, :], ot[:])

    _split_multi_waits(nc)
    return nc


_NC_CACHE = []


def kernel(**inputs) -> np.ndarray:
    import ml_dtypes

    x = np.asarray(inputs["history_in"], dtype=np.float32)     # [B, T, F]
    wtr = np.asarray(inputs["trend_W"], dtype=np.float32)      # [F, T, N]
    wre = np.asarray(inputs["residual_W"], dtype=np.float32)   # [F, T, N]
    tb = np.asarray(inputs["trend_b"], dtype=np.float32)       # [F, N]
    rb = np.asarray(inputs["residual_b"], dtype=np.float32)    # [F, N]

    # fold trend+residual GEMMs into one effective weight (fp32 math,
    # single bf16 rounding at the end)
    v = (wtr - wre) * 0.5
    weff = (wtr + wre) * 0.5
    weff[:, 1:, :] += v[:, :-1, :]
    weff[:, T - 1, :] += v[:, T - 1, :]

    # partition-major repacks (see _build docstring)
    xP = np.ascontiguousarray(
        x.transpose(2, 1, 0).reshape(F, TC, 128, B).transpose(0, 2, 1, 3)
    ).astype(ml_dtypes.bfloat16)                               # [F,128,TC,B]
    wP = np.ascontiguousarray(
        weff.reshape(F, TC, 128, N).transpose(0, 2, 1, 3)
    ).astype(ml_dtypes.bfloat16)                               # [F,128,TC,N]

    if not _NC_CACHE:
        _NC_CACHE.append(_build())
    nc = _NC_CACHE[0]

    in_maps = []
    for k in range(NCORES):
        sl = slice(FL * k, FL * (k + 1))
        in_maps.append({
            "xP": np.ascontiguousarray(xP[sl]),
            "wP": np.ascontiguousarray(wP[sl]),
        })

    res = run_bass_kernel_spmd(nc, in_maps, core_ids=list(range(NCORES)))
    full = np.concatenate(
        [np.asarray(r["out"]) for r in res.results], axis=0)   # [F, B, N] bf16
    out = full.astype(np.float32).transpose(1, 2, 0)           # [B, N, F]
    out += (tb + rb).T[None, :, :]                             # host bias
    return np.ascontiguousarray(out)
